# revision 13
# baseline (speedup 1.0000x reference)
"""PhysicsCrossAttention fused kernel for 8 Trainium2 NeuronCores.

Strategy: pure data-parallel over batch (2048 rows/core). All activations are
kept in transposed layout [feature, batch] (feature on SBUF partitions), so
every matmul chains with weights as the stationary operand and no on-chip
transposes are ever needed. Host pre-transposes inputs / re-transposes the
output, pre-lays-out weights in SBUF tile order, and folds LayerNorm gains,
the 1/sqrt(dh) score scale, and sigmoid(gate) into the weights.

LayerNorm is handled with augmented matmuls:
  LN(x)@W + b = invs * (x@W' - mu (x) u + v (x) sigma)      (per batch column)
where W' = diag(g)W, u = colsum(W'), v = b_ln@W + b are host-folded; the
-u (x) mu + v (x) sigma rank-2 term is a K=2 matmul accumulated into the same
PSUM tile, and the per-column invs scale is applied by multiplying with a
broadcast matrix built by a K=1 ones-matmul.
"""

from contextlib import ExitStack

import numpy as np
import ml_dtypes

import concourse.bass as bass
import concourse.tile as tile
from concourse import bacc, mybir
from concourse.bass_utils import run_bass_kernel_spmd

BF = ml_dtypes.bfloat16
bf16 = mybir.dt.bfloat16
f32 = mybir.dt.float32
AF = mybir.ActivationFunctionType
GELU_AF = AF.Gelu_apprx_tanh  # test_sim swaps this for a sim-supported fn

B, D, PHYS, NT, NH = 16384, 1024, 11, 4, 4
DH = D // NH
DN = D * NT
EPS = 1e-5
NCORES = 8
BC_FULL = B // NCORES   # 2048 rows per core
BT = 512                # batch tile (matmul moving free dim)

KD = D // 128           # 8   k-chunks for D-contractions
ND = D // 128           # 8   n-chunks for D-wide outputs
NF = 2 * D // 128       # 16  n-chunks for FFN hidden
N2 = DN // 128          # 32  n-chunks (and k-chunks) for the 4096 dims


def build_program(bc):
    """Emit the per-core program (SPMD: same program, different data)."""
    nbt = bc // BT
    nc = bacc.Bacc("TRN2", target_bir_lowering=False, debug=False,
                   num_devices=NCORES)

    def din(name, shape, dt):
        return nc.dram_tensor(name, shape, dt, kind="ExternalInput")

    embbf_d = din("embbf", [D, bc], bf16)     # raw embedding^T (LN input)
    embp_d = din("embp", [D, bc], f32)        # (embedding + sg*bo)^T residual
    phys_d = din("phys", [PHYS, bc], bf16)
    pw1_d = din("pw1", [PHYS, N2, 128], bf16)
    pb1_d = din("pb1", [128, N2], f32)
    pw2_d = din("pw2", [N2, 128, N2, 128], bf16)   # [nn, p, kc, f]
    pb2_d = din("pb2", [128, N2], f32)
    wq_d = din("wq", [128, KD, ND, 128], bf16)
    wk_d = din("wk", [128, KD, ND, 128], bf16)
    wv_d = din("wv", [128, KD, ND, 128], bf16)
    wo_d = din("wo", [128, KD, ND, 128], bf16)
    fw1a_d = din("fw1a", [128, 4, NF, 128], bf16)  # k-chunks 0..3
    fw1b_d = din("fw1b", [128, 4, NF, 128], bf16)  # k-chunks 4..7
    fw2a_d = din("fw2a", [128, 8, ND, 128], bf16)  # k-chunks 0..7
    fw2b_d = din("fw2b", [128, 8, ND, 128], bf16)  # k-chunks 8..15
    aq_d = din("aq", [2, ND, 128], bf16)           # rows: -uq, vq
    ak_d = din("ak", [66, ND, 128], bf16)          # rows 32b/32b+1: -uk, vk
    av_d = din("av", [66, ND, 128], bf16)
    af_d = din("af", [2, NF, 128], bf16)
    fb2a_d = din("fb2a", [1, ND, 128], bf16)
    ones_d = din("ones", [128, BT], bf16)
    outT_d = nc.dram_tensor("outT", [D, bc], f32, kind="ExternalOutput")
    phi_d = nc.dram_tensor("phi", [N2, nbt, 128, BT], bf16)       # scratch
    fused_d = nc.dram_tensor("fusedsc", [ND, nbt, 128, BT], f32)  # scratch

    with tile.TileContext(nc) as tc:
        cpool = tc.alloc_tile_pool(name="const", bufs=1)
        kvpool = tc.alloc_tile_pool(name="kvstat", bufs=1)
        finp = tc.alloc_tile_pool(name="fin", bufs=1)
        augqf = tc.alloc_tile_pool(name="augqf", bufs=1)
        cstack = [cpool, kvpool, finp, augqf]

        ones_t = cpool.tile([128, BT], bf16)
        nc.sync.dma_start(ones_t[:], ones_d[:])
        aq_t = cpool.tile([2, ND, 128], bf16)
        nc.sync.dma_start(aq_t[:], aq_d[:])
        ak_t = cpool.tile([66, ND, 128], bf16)
        nc.sync.dma_start(ak_t[:], ak_d[:])
        av_t = cpool.tile([66, ND, 128], bf16)
        nc.sync.dma_start(av_t[:], av_d[:])
        af_t = cpool.tile([2, NF, 128], bf16)
        nc.sync.dma_start(af_t[:], af_d[:])
        fb2a_t = cpool.tile([1, ND, 128], bf16)
        nc.sync.dma_start(fb2a_t[:], fb2a_d[:])
        eps_t = cpool.tile([128, 1], f32, name="epsc")
        nc.gpsimd.memset(eps_t[:], EPS)

        # packed per-token LN stats, written in phase 2, read in phase 3
        # (rows 32b hold batch-tile b; row 32b+1 of aug holds sigma)
        augkv = [kvpool.tile([66, BT], bf16, tag=f"augkv{t}", name=f"augkv{t}")
                 for t in range(NT)]
        invskv = [kvpool.tile([65, BT], bf16, tag=f"invskv{t}",
                              name=f"invskv{t}") for t in range(NT)]
        # PE row-group 3 (partitions 96+) is unusable for matmul operands on
        # TRN2, so batch-tile 3 gets dedicated base-0 tiles instead.
        augkv3 = [kvpool.tile([2, BT], bf16, tag=f"augkv3{t}",
                              name=f"augkv3{t}") for t in range(NT)]
        invskv3 = [kvpool.tile([1, BT], bf16, tag=f"invskv3{t}",
                               name=f"invskv3{t}") for t in range(NT)]

        def kv_slot(t, b):
            if b < 3:
                return augkv[t], invskv[t], 32 * b
            return augkv3[t], invskv3[t], 0

        def finalize_stats(st, dn, packed=None):
            """st: PSUM [33,BT], row0=sum(x), row32=sum(x^2) over dn elems.
            packed=(aug_tile, invs_tile, row): write mu/sigma/invs there and
            return None; else return (aug [2,BT] bf16, invs_bf [1,BT] ap).
            ff lanes: 0=mu, 1=m2 then inv, 2=musq then sigma, 3=var."""
            ff = finp.tile([1, 4 * BT], f32, tag="fin_f", name="ff")
            L = lambda i: ff[0:1, i * BT:(i + 1) * BT]
            nc.scalar.activation(L(0), st[0:1, :], AF.Copy, scale=1.0 / dn)
            nc.scalar.activation(L(1), st[32:33, :], AF.Copy, scale=1.0 / dn)
            nc.vector.tensor_mul(L(2), L(0), L(0))
            nc.vector.tensor_sub(L(3), L(1), L(2))
            nc.scalar.activation(L(2), L(3), AF.Sqrt, bias=eps_t[0:1, :])
            nc.vector.reciprocal(L(1), L(2))
            fb = finp.tile([1, 3 * BT], bf16, tag="fin_b", name="fb")
            M = lambda i: fb[0:1, i * BT:(i + 1) * BT]
            nc.scalar.copy(M(0), L(0))   # mu
            nc.scalar.copy(M(1), L(2))   # sigma
            if packed is not None:
                augt, invst, row = packed
                nc.sync.dma_start(augt[row:row + 1, :], M(0))
                nc.sync.dma_start(augt[row + 1:row + 2, :], M(1))
                nc.scalar.copy(invst[row:row + 1, :], L(1))
                return None
            aug = augqf.tile([2, BT], bf16, tag="aug", name="aug")
            nc.sync.dma_start(aug[0:1, :], M(0))
            nc.sync.dma_start(aug[1:2, :], M(1))
            nc.scalar.copy(M(2), L(1))   # inv (bf16)
            return aug, M(2)

        # ---------------- phase 1 + 2: phys MLP ----------------
        p12pools = []

        def _q(name, bufs, **kw):
            p = tc.alloc_tile_pool(name=name, bufs=bufs, **kw)
            p12pools.append(p)
            return p

        p12w = _q("p12w", 1)
        hpool = _q("hpool", 1)
        pw2p = _q("pw2p", 2)
        phidr = _q("phidr", 3)
        sqp2 = _q("sqp2", 3)
        p12 = _q("p12", 3, space="PSUM")
        st2 = _q("st2", max(nbt, 2), space="PSUM")

        phys_t = p12w.tile([PHYS, bc], bf16)
        nc.sync.dma_start(phys_t[:], phys_d[:])
        pw1_t = p12w.tile([PHYS, N2, 128], bf16)
        nc.sync.dma_start(pw1_t[:], pw1_d[:])
        pb1_t = p12w.tile([128, N2], f32)
        nc.sync.dma_start(pb1_t[:], pb1_d[:])
        pb2_t = p12w.tile([128, N2], f32)
        nc.sync.dma_start(pb2_t[:], pb2_d[:])

        h_t = hpool.tile([128, N2, bc], bf16)
        for b in range(nbt):
            bs = slice(b * BT, (b + 1) * BT)
            for kc in range(N2):
                ps = p12.tile([128, BT], f32, name="ps1", tag="p12")
                nc.tensor.matmul(ps[:], pw1_t[:, kc, :], phys_t[:, bs],
                                 start=True, stop=True)
                nc.scalar.activation(h_t[:, kc, bs], ps[:],
                                     GELU_AF,
                                     bias=pb1_t[:, kc:kc + 1])

        st2_tiles = [None] * nbt
        for nn in range(N2):
            w2 = pw2p.tile([128, N2, 128], bf16, name="w2", tag="w2")
            nc.sync.dma_start(w2[:], pw2_d[nn])
            t = nn // (N2 // NT)
            first = nn % (N2 // NT) == 0
            last = nn % (N2 // NT) == (N2 // NT) - 1
            for b in range(nbt):
                bs = slice(b * BT, (b + 1) * BT)
                ps = p12.tile([128, BT], f32, name="ps2", tag="p12")
                for k in range(N2):
                    nc.tensor.matmul(ps[:], w2[:, k, :], h_t[:, k, bs],
                                     start=(k == 0), stop=(k == N2 - 1))
                phit = phidr.tile([128, BT], bf16, name="phit", tag="phit")
                nc.scalar.activation(phit[:], ps[:], AF.Identity,
                                     bias=pb2_t[:, nn:nn + 1])
                nc.sync.dma_start(phi_d[nn, b], phit[:])
                if first:
                    st2_tiles[b] = st2.tile([33, BT], f32, name="st2t",
                                            tag="st2t")
                stt = st2_tiles[b]
                nc.tensor.matmul(stt[0:1, :], ones_t[:, 0:1], phit[:],
                                 start=first, stop=last,
                                 skip_group_check=True)
                sq = sqp2.tile([128, BT], bf16, name="sq2", tag="sq2")
                nc.scalar.activation(sq[:], phit[:], AF.Square)
                nc.tensor.matmul(stt[32:33, :], ones_t[:, 0:1], sq[:],
                                 start=first, stop=last,
                                 skip_group_check=True)
                if last:
                    finalize_stats(stt, float(D), packed=kv_slot(t, b))

        for p in reversed(p12pools):
            p.release()

        # ---------------- phase 3: attention + FFN ----------------
        p3pools = []

        def _p(name, bufs, **kw):
            p = tc.alloc_tile_pool(name=name, bufs=bufs, **kw)
            p3pools.append(p)
            return p

        phibp = _p("phibp", 1)
        embbfp = _p("embbfp", 1)
        embpp = _p("embpp", 1)
        qhp = _p("qhp", 1)
        kvp = _p("kvp", 1)
        arepp = _p("arepp", 5)
        scrp = _p("scrp", 2)
        ctxp = _p("ctxp", 1)
        accp = _p("accp", 3)
        fusedbfp = _p("fusedbfp", 1)
        ftmp = _p("ftmp", 1)
        outp = _p("outp", 2)
        repp = _p("repp", 6)
        smxp = _p("smxp", 1)
        wqop = _p("wqop", 1)
        wkp = _p("wkp", 1)
        wvp = _p("wvp", 1)
        ps3 = _p("ps3", 4, space="PSUM")
        st3 = _p("st3", 1, space="PSUM")
        scps = _p("scps", 2, space="PSUM")

        embbf_ap = embbf_d.ap().rearrange("(a p) c -> p a c", p=128)
        phi_ap = phi_d.ap()

        for b in range(nbt):
            bs = slice(b * BT, (b + 1) * BT)
            phib = phibp.tile([128, N2, BT], bf16, tag="phib", name="phib")
            nc.sync.dma_start(
                phib[:], phi_ap[:, b].rearrange("a p c -> p a c"))
            embbf = embbfp.tile([128, KD, BT], bf16, tag="embbf", name="embbf")
            nc.sync.dma_start(embbf[:], embbf_ap[:, :, bs])

            # rebuild per-(token,b) invs broadcast matrices
            kvrep = []
            for t in range(NT):
                _, invst, row = kv_slot(t, b)
                rp = ps3.tile([128, BT], f32, name="rp", tag="ps3")
                nc.tensor.matmul(rp[:], ones_t[row:row + 1, 0:128],
                                 invst[row:row + 1, :],
                                 start=True, stop=True,
                                 tile_position=(row, 0))
                kr = repp.tile([128, BT], bf16, tag="rep", name="kr")
                nc.vector.tensor_copy(kr[:], rp[:])
                kvrep.append(kr)

            # LN-q stats
            stq = st3.tile([33, BT], f32, name="stq", tag="st3")
            for kc in range(KD):
                sq = scrp.tile([128, BT], bf16, name="sq3", tag="scr")
                nc.scalar.activation(sq[:], embbf[:, kc, :], AF.Square)
                nc.tensor.matmul(stq[0:1, :], ones_t[:, 0:1],
                                 embbf[:, kc, :], start=(kc == 0),
                                 stop=(kc == KD - 1), skip_group_check=True)
                nc.tensor.matmul(stq[32:33, :], ones_t[:, 0:1], sq[:],
                                 start=(kc == 0), stop=(kc == KD - 1),
                                 skip_group_check=True)
            aug_q, invq_b = finalize_stats(stq, float(D))
            rp = ps3.tile([128, BT], f32, name="rpq", tag="ps3")
            nc.tensor.matmul(rp[:], ones_t[0:1, 0:128], invq_b,
                             start=True, stop=True)
            invq_rep = repp.tile([128, BT], bf16, tag="rep", name="invq_rep")
            nc.vector.tensor_copy(invq_rep[:], rp[:])

            # q projection (LN folded)
            wq_t = wqop.tile([128, KD, ND, 128], bf16, tag="wqo", name="wq_t")
            nc.sync.dma_start(wq_t[:], wq_d[:])
            qh = qhp.tile([128, ND, BT], bf16, tag="qh", name="qh")
            for nn in range(ND):
                ps = ps3.tile([128, BT], f32, name="psq", tag="ps3")
                for k in range(KD):
                    nc.tensor.matmul(ps[:], wq_t[:, k, nn, :],
                                     embbf[:, k, :], start=(k == 0),
                                     stop=False)
                nc.tensor.matmul(ps[:], aq_t[:, nn, :], aug_q[:],
                                 start=False, stop=True)
                nc.vector.tensor_mul(qh[:, nn, :], ps[:], invq_rep[:])

            # attention, one head at a time
            wk_t = wkp.tile([128, KD, ND, 128], bf16, tag="wk", name="wk_t")
            nc.sync.dma_start(wk_t[:], wk_d[:])
            wv_t = wvp.tile([128, KD, ND, 128], bf16, tag="wv", name="wv_t")
            nc.sync.dma_start(wv_t[:], wv_d[:])
            ctx = ctxp.tile([128, ND, BT], bf16, tag="ctx", name="ctx")
            for h in range(NH):
                kh = kvp.tile([128, 2 * NT, BT], bf16, tag="kh", name="kh")
                vh = kvp.tile([128, 2 * NT, BT], bf16, tag="vh", name="vh")
                for t in range(NT):
                    for dc in range(2):
                        nn = 2 * h + dc
                        for dst, wt, at in ((kh, wk_t, ak_t),
                                            (vh, wv_t, av_t)):
                            ps = ps3.tile([128, BT], f32, name="pskv",
                                          tag="ps3")
                            for k in range(KD):
                                nc.tensor.matmul(
                                    ps[:], wt[:, k, nn, :],
                                    phib[:, (N2 // NT) * t + k, :],
                                    start=(k == 0), stop=False)
                            augt, _, row = kv_slot(t, b)
                            nc.tensor.matmul(
                                ps[:], at[row:row + 2, nn, :],
                                augt[row:row + 2, :],
                                start=False, stop=True,
                                tile_position=(row, 0))
                            nc.vector.tensor_mul(dst[:, 2 * t + dc, :],
                                                 ps[:], kvrep[t][:])
                # scores + softmax (scale 1/16 folded into wq)
                es = smxp.tile([1, NT * BT], bf16, tag="es", name="es")
                for t in range(NT):
                    sc = scps.tile([1, BT], f32, name="sc", tag="scps")
                    for dc in range(2):
                        prod = scrp.tile([128, BT], bf16, name="prod",
                                         tag="scr")
                        nc.vector.tensor_mul(prod[:], qh[:, 2 * h + dc, :],
                                             kh[:, 2 * t + dc, :])
                        nc.tensor.matmul(sc[:], ones_t[:, 0:1], prod[:],
                                         start=(dc == 0), stop=(dc == 1))
                    nc.scalar.activation(
                        es[0:1, t * BT:(t + 1) * BT], sc[:], AF.Exp)
                sf = smxp.tile([1, 3 * BT], f32, tag="sf", name="sf")
                nc.vector.tensor_add(sf[0:1, 0:BT], es[0:1, 0:BT],
                                     es[0:1, BT:2 * BT])
                nc.vector.tensor_add(sf[0:1, BT:2 * BT],
                                     es[0:1, 2 * BT:3 * BT],
                                     es[0:1, 3 * BT:4 * BT])
                nc.vector.tensor_add(sf[0:1, 2 * BT:3 * BT],
                                     sf[0:1, 0:BT], sf[0:1, BT:2 * BT])
                # r overwrites the (now dead) first partial sum
                nc.vector.reciprocal(sf[0:1, 0:BT], sf[0:1, 2 * BT:3 * BT])
                rb = smxp.tile([1, BT], bf16, tag="rb", name="rb")
                nc.scalar.copy(rb[:], sf[0:1, 0:BT])
                # replicate e_t and 1/den down 128 partitions
                areps = []
                for t in range(NT):
                    rp = ps3.tile([128, BT], f32, name="rpe", tag="ps3")
                    nc.tensor.matmul(rp[:], ones_t[0:1, 0:128],
                                     es[0:1, t * BT:(t + 1) * BT],
                                     start=True, stop=True)
                    ar = arepp.tile([128, BT], bf16, tag="arep", name="ar")
                    nc.vector.tensor_copy(ar[:], rp[:])
                    areps.append(ar)
                rp = ps3.tile([128, BT], f32, name="rpr", tag="ps3")
                nc.tensor.matmul(rp[:], ones_t[0:1, 0:128], rb[:],
                                 start=True, stop=True)
                rrep = arepp.tile([128, BT], bf16, tag="arep", name="rrep")
                nc.vector.tensor_copy(rrep[:], rp[:])
                for dc in range(2):
                    a0 = accp.tile([128, BT], f32, tag="acc", name="a0")
                    nc.vector.tensor_mul(a0[:], areps[0][:], vh[:, 0 + dc, :])
                    a1 = accp.tile([128, BT], f32, tag="acc", name="a1")
                    nc.vector.tensor_mul(a1[:], areps[1][:], vh[:, 2 + dc, :])
                    s0 = accp.tile([128, BT], f32, tag="acc", name="s0")
                    nc.vector.tensor_add(s0[:], a0[:], a1[:])
                    a2 = accp.tile([128, BT], f32, tag="acc", name="a2")
                    nc.vector.tensor_mul(a2[:], areps[2][:], vh[:, 4 + dc, :])
                    s1 = accp.tile([128, BT], f32, tag="acc", name="s1")
                    nc.vector.tensor_add(s1[:], s0[:], a2[:])
                    a3 = accp.tile([128, BT], f32, tag="acc", name="a3")
                    nc.vector.tensor_mul(a3[:], areps[3][:], vh[:, 6 + dc, :])
                    s2 = accp.tile([128, BT], f32, tag="acc", name="s2")
                    nc.vector.tensor_add(s2[:], s1[:], a3[:])
                    nc.vector.tensor_mul(ctx[:, 2 * h + dc, :], s2[:],
                                         rrep[:])

            # output projection + residual (bo*sg folded into embp)
            wo_t = wqop.tile([128, KD, ND, 128], bf16, tag="wqo", name="wo_t")
            nc.sync.dma_start(wo_t[:], wo_d[:])
            fusedbf = fusedbfp.tile([128, ND, BT], bf16, tag="fusedbf",
                                    name="fusedbf")
            stf = st3.tile([33, BT], f32, name="stf", tag="st3")
            for nn in range(ND):
                ps = ps3.tile([128, BT], f32, name="pso", tag="ps3")
                for k in range(KD):
                    nc.tensor.matmul(ps[:], wo_t[:, k, nn, :],
                                     ctx[:, k, :], start=(k == 0),
                                     stop=(k == KD - 1))
                ep = embpp.tile([128, BT], f32, tag="embp", name="ep")
                nc.sync.dma_start(
                    ep[:], embp_d[nn * 128:(nn + 1) * 128, bs])
                fs = ftmp.tile([128, BT], f32, tag="ftmp", name="fs")
                nc.vector.tensor_add(fs[:], ps[:], ep[:])
                nc.sync.dma_start(fused_d[nn, b], fs[:])
                nc.scalar.copy(fusedbf[:, nn, :], fs[:])
                sq = scrp.tile([128, BT], bf16, name="sqf", tag="scr")
                nc.scalar.activation(sq[:], fusedbf[:, nn, :], AF.Square)
                nc.tensor.matmul(stf[0:1, :], ones_t[:, 0:1],
                                 fusedbf[:, nn, :], start=(nn == 0),
                                 stop=(nn == ND - 1), skip_group_check=True)
                nc.tensor.matmul(stf[32:33, :], ones_t[:, 0:1], sq[:],
                                 start=(nn == 0), stop=(nn == ND - 1),
                                 skip_group_check=True)
            aug_f, invf_b = finalize_stats(stf, float(D))
            rp = ps3.tile([128, BT], f32, name="rpf", tag="ps3")
            nc.tensor.matmul(rp[:], ones_t[0:1, 0:128], invf_b,
                             start=True, stop=True)
            invf_rep = repp.tile([128, BT], bf16, tag="rep", name="invf_rep")
            nc.vector.tensor_copy(invf_rep[:], rp[:])

            # FFN (LN folded); fw1 reuses the wk/wv slots, fw2 the wqo slot
            fw1a_t = wkp.tile([128, 4, NF, 128], bf16, tag="wk", name="fw1a_t")
            nc.sync.dma_start(fw1a_t[:], fw1a_d[:])
            fw1b_t = wvp.tile([128, 4, NF, 128], bf16, tag="wv", name="fw1b_t")
            nc.sync.dma_start(fw1b_t[:], fw1b_d[:])
            g1a = kvp.tile([128, NT * 2, BT], bf16, tag="kh", name="g1a")
            g1b = kvp.tile([128, NT * 2, BT], bf16, tag="vh", name="g1b")
            for nn in range(NF):
                ps = ps3.tile([128, BT], f32, name="psf1", tag="ps3")
                for k in range(KD):
                    wt = fw1a_t[:, k, nn, :] if k < 4 \
                        else fw1b_t[:, k - 4, nn, :]
                    nc.tensor.matmul(ps[:], wt, fusedbf[:, k, :],
                                     start=(k == 0), stop=False)
                nc.tensor.matmul(ps[:], af_t[:, nn, :], aug_f[:],
                                 start=False, stop=True)
                z1 = scrp.tile([128, BT], bf16, name="z1", tag="scr")
                nc.vector.tensor_mul(z1[:], ps[:], invf_rep[:])
                g1 = g1a if nn < 8 else g1b
                nc.scalar.activation(g1[:, nn % 8, :], z1[:],
                                     GELU_AF)
            fw2a_t = wqop.tile([128, 8, ND, 128], bf16, tag="wqo",
                               name="fw2a_t")
            nc.sync.dma_start(fw2a_t[:], fw2a_d[:])
            fw2b_t = wkp.tile([128, 8, ND, 128], bf16, tag="wk", name="fw2b_t")
            nc.sync.dma_start(fw2b_t[:], fw2b_d[:])
            for nn in range(ND):
                ps = ps3.tile([128, BT], f32, name="psf2", tag="ps3")
                for k in range(2 * KD):
                    wt = fw2a_t[:, k, nn, :] if k < 8 \
                        else fw2b_t[:, k - 8, nn, :]
                    g1 = g1a if k < 8 else g1b
                    nc.tensor.matmul(ps[:], wt, g1[:, k % 8, :],
                                     start=(k == 0), stop=False)
                nc.tensor.matmul(ps[:], fb2a_t[0:1, nn, :],
                                 ones_t[0:1, :], start=False, stop=True)
                fr = ftmp.tile([128, BT], f32, tag="ftmp", name="fr")
                nc.sync.dma_start(fr[:], fused_d[nn, b])
                ot = outp.tile([128, BT], f32, tag="out", name="ot")
                nc.vector.tensor_add(ot[:], ps[:], fr[:])
                nc.sync.dma_start(
                    outT_d[nn * 128:(nn + 1) * 128, bs], ot[:])

        for p in reversed(p3pools):
            p.release()
        for p in reversed(cstack):
            p.release()

    nc.compile()
    return nc


def prep_inputs(inputs, bc):
    """Host-side: fold LN gains / gate / score scale into weights, lay out
    weights in SBUF tile order, shard + transpose activations."""
    f = lambda k: np.asarray(inputs[k], np.float32)
    emb, phys = f("embedding"), f("physics")
    pw1, pb1, pw2, pb2 = f("pw1"), f("pb1"), f("pw2"), f("pb2")
    lnq_g, lnq_b = f("lnq_g"), f("lnq_b")
    lnkv_g, lnkv_b = f("lnkv_g"), f("lnkv_b")
    wq, bq, wk, bk = f("wq"), f("bq"), f("wk"), f("bk")
    wv, bv, wo, bo = f("wv"), f("bv"), f("wo"), f("bo")
    ffn_g, ffn_b = f("ffn_g"), f("ffn_b")
    fw1, fb1, fw2, fb2 = f("fw1"), f("fb1"), f("fw2"), f("fb2")
    gate = f("gate")

    sg = 1.0 / (1.0 + np.exp(-gate[0]))
    sc = 1.0 / np.sqrt(DH)

    wq_ = (lnq_g[:, None] * wq) * sc
    uq = (lnq_g @ wq) * sc
    vq = (lnq_b @ wq + bq) * sc
    wk_ = lnkv_g[:, None] * wk
    uk = lnkv_g @ wk
    vk = lnkv_b @ wk + bk
    wv_ = lnkv_g[:, None] * wv
    uv = lnkv_g @ wv
    vv = lnkv_b @ wv + bv
    wo_ = sg * wo
    fw1_ = ffn_g[:, None] * fw1
    uf = ffn_g @ fw1
    vf = ffn_b @ fw1 + fb1
    embp = emb + sg * bo

    def wlay(w, kc, nn):
        return np.ascontiguousarray(
            w.reshape(kc, 128, nn, 128).transpose(1, 0, 2, 3)).astype(BF)

    def aug2(u, v, nch):
        return np.ascontiguousarray(
            np.stack([-u.reshape(nch, 128), v.reshape(nch, 128)])).astype(BF)

    def aug98(u, v, nch):
        a = np.zeros((66, nch, 128), np.float32)
        for bb in range(3):
            a[32 * bb] = -u.reshape(nch, 128)
            a[32 * bb + 1] = v.reshape(nch, 128)
        return a.astype(BF)

    fw1l = wlay(fw1_, 8, NF)
    fw2l = wlay(fw2, 16, ND)
    shared = {
        "pw1": np.ascontiguousarray(pw1.reshape(PHYS, N2, 128)).astype(BF),
        "pb1": np.ascontiguousarray(pb1.reshape(N2, 128).T),
        "pw2": np.ascontiguousarray(
            pw2.reshape(N2, 128, N2, 128).transpose(2, 1, 0, 3)).astype(BF),
        "pb2": np.ascontiguousarray(pb2.reshape(N2, 128).T),
        "wq": wlay(wq_, KD, ND), "wk": wlay(wk_, KD, ND),
        "wv": wlay(wv_, KD, ND), "wo": wlay(wo_, KD, ND),
        "fw1a": np.ascontiguousarray(fw1l[:, 0:4]),
        "fw1b": np.ascontiguousarray(fw1l[:, 4:8]),
        "fw2a": np.ascontiguousarray(fw2l[:, 0:8]),
        "fw2b": np.ascontiguousarray(fw2l[:, 8:16]),
        "aq": aug2(uq, vq, ND),
        "ak": aug98(uk, vk, ND),
        "av": aug98(uv, vv, ND),
        "af": aug2(uf, vf, NF),
        "fb2a": np.ascontiguousarray(fb2.reshape(ND, 128))[None].astype(BF),
        "ones": np.ones((128, BT), BF),
    }
    ncores = emb.shape[0] // bc
    in_maps = []
    for c in range(ncores):
        sl = slice(c * bc, (c + 1) * bc)
        m = dict(shared)
        m["embbf"] = np.ascontiguousarray(emb[sl].T).astype(BF)
        m["embp"] = np.ascontiguousarray(embp[sl].T)
        m["phys"] = np.ascontiguousarray(phys[sl].T).astype(BF)
        in_maps.append(m)
    return in_maps


_NC_CACHE = {}


def kernel(**inputs) -> np.ndarray:
    bc = BC_FULL
    if bc not in _NC_CACHE:
        _NC_CACHE[bc] = build_program(bc)
    nc = _NC_CACHE[bc]
    in_maps = prep_inputs(inputs, bc)
    res = run_bass_kernel_spmd(nc, in_maps, list(range(NCORES)))
    out = np.empty((B, D), np.float32)
    for c in range(NCORES):
        out[c * bc:(c + 1) * bc] = res.results[c]["outT"].T
    return out
